# revision 6
# baseline (speedup 1.0000x reference)
"""Batched GAT kernel for 8 Trainium2 NeuronCores.

Math: out[b,i,:] = softmax_j(mask(leakyrelu(s_i+t_j))) @ h  per head, concat heads.

Decomposition (avoids exp on the [i,j] pairwise data entirely):
  exp(lrelu(e)) = max(u_i*v_j, u'_i*v'_j)   u=exp(s), v=exp(t), u'=exp(.2s), v'=exp(.2t)
                = G*(u v) + (1-G)*(u' v'),  G = 1[e>=0]
  p = m * that, m = 1[adj>0.5]
  num_f = u_i*(Gm @ v.h)_f + u'_i*((M @ v'.h)_f - (Gm @ v'.h)_f);  den analogous.

Per core (c = 0..7): b = c//2, rows i in [ (c%2)*1024, +1024 ).
Gm is built per (head, j-tile) on DVE: tensor_scalar (s_bcast >= -t_col) at 4x,
then tensor_tensor * mT at 2x (a few j-tiles offloaded to GPSIMD).
PE consumes Gm/M as fp16 rhs against fp16 value-packs; mask transpose is done
on PE (128x128 fp16 transpose-matmuls), NOT DMA-xbar (HWDGE per-instruction
overhead serializes, and cross-queue xbar transpose/copy races corrupt data).
"""
import os
import sys
import numpy as np

for _p in ("/opt/trn_rl_repo",):
    if _p not in sys.path:
        sys.path.insert(0, _p)

B, N, D, H, F = 4, 2048, 128, 4, 32
HF = H * F           # 128
IR = 1024            # i-rows per core
NJT = N // 128       # 16 j-tiles
NCORES = 8

_CACHE = {}


def build_nc(reps=1):
    import concourse.bacc as bacc
    import concourse.tile as tile
    from concourse import mybir

    f32, f16 = mybir.dt.float32, mybir.dt.float16
    Alu = mybir.AluOpType
    Act = mybir.ActivationFunctionType

    nc = bacc.Bacc(None, target_bir_lowering=False)

    xT_d   = nc.dram_tensor("xT",   [D, N],    f32, kind="ExternalInput")
    xiT_d  = nc.dram_tensor("xiT",  [D, IR],   f32, kind="ExternalInput")
    adj_d  = nc.dram_tensor("adjS", [IR, N],   f32, kind="ExternalInput")
    Wf_d   = nc.dram_tensor("Wf",   [D, HF],   f32, kind="ExternalInput")
    aS_d   = nc.dram_tensor("aS",   [HF, H],   f32, kind="ExternalInput")
    aD_d   = nc.dram_tensor("aD",   [HF, H],   f32, kind="ExternalInput")
    bias_d = nc.dram_tensor("biasC", [HF, 1],  f32, kind="ExternalInput")
    out_d  = nc.dram_tensor("out",  [IR, HF],  f32, kind="ExternalOutput")

    # host constants
    S4 = np.zeros((128, 4), np.float32)
    B4 = np.zeros((4, 128), np.float32)
    for h in range(H):
        S4[h * 32, h] = 1.0
        B4[h, h * 32:(h + 1) * 32] = 1.0
    EY = np.zeros((4, 4 * 128), np.float16)
    for h in range(H):
        EY[h, h * 128:(h + 1) * 128] = 1.0
    S4_d = nc.inline_tensor(S4, "S4c")
    B4_d = nc.inline_tensor(B4, "B4c")
    EY_d = nc.inline_tensor(EY, "EYc")
    ID_d = nc.inline_tensor(np.eye(128, dtype=np.float32), "identc")
    ID16_d = nc.inline_tensor(np.eye(128, dtype=np.float16), "ident16c")

    with tile.TileContext(nc) as tc:
        cst_ctx = tc.tile_pool(name="cst", bufs=1)
        cst = cst_ctx.__enter__()
        try:
            # ---------------- persistent tiles ----------------
            prep_ctx = tc.tile_pool(name="prep", bufs=1)
            prep = prep_ctx.__enter__()
            xT   = prep.tile([D, N], f32)
            xiT  = prep.tile([D, IR], f32)
            hT   = prep.tile([HF, N], f32)
            hiT  = prep.tile([HF, IR], f32)
            h_sb = prep.tile([128, NJT, HF], f32)

            Wf   = cst.tile([D, HF], f32)
            aS   = cst.tile([HF, H], f32)
            aD   = cst.tile([HF, H], f32)
            biasC = cst.tile([HF, 1], f32)
            s4c  = cst.tile([128, 4], f32)
            b4c  = cst.tile([4, 128], f32)
            eyc  = cst.tile([4, 4 * 128], f16)
            idc  = cst.tile([128, 128], f32)
            idc16 = cst.tile([128, 128], f16)

            nc.sync.dma_start(xT[:], xT_d[:])
            nc.sync.dma_start(xiT[:], xiT_d[:])
            nc.sync.dma_start(Wf[:], Wf_d[:])
            nc.sync.dma_start(aS[:], aS_d[:])
            nc.sync.dma_start(aD[:], aD_d[:])
            nc.sync.dma_start(biasC[:], bias_d[:])
            nc.sync.dma_start(s4c[:], S4_d[:])
            nc.sync.dma_start(b4c[:], B4_d[:])
            nc.sync.dma_start(eyc[:], EY_d[:])
            nc.sync.dma_start(idc[:], ID_d[:])
            nc.sync.dma_start(idc16[:], ID16_d[:])

            sZ4   = cst.tile([4, IR], f32)
            u1r4  = cst.tile([4, IR], f32)
            u02r4 = cst.tile([4, IR], f32)
            s16_4 = cst.tile([4, IR], f16)
            negt  = cst.tile([128, NJT, H], f32)
            v1c   = cst.tile([128, NJT, H], f32)
            v02c  = cst.tile([128, NJT, H], f32)
            sb16  = cst.tile([128, H, IR], f16)
            u1b   = cst.tile([128, IR], f32)
            u02b  = cst.tile([128, IR], f32)
            Gpack = cst.tile([128, NJT, H, 128], f16)
            Mpack = cst.tile([128, NJT, HF], f16)
            MdenP = cst.tile([128, NJT, HF], f16)
            mT_all = cst.tile([128, NJT, IR], f16)

            As_all = cst.tile([128, IR], f32)
            Cs_all = cst.tile([128, IR], f32)
            DenA   = cst.tile([128, IR], f32)
            DenC   = cst.tile([128, IR], f32)
            Ms_sb  = cst.tile([128, IR], f32)
            Mds_sb = cst.tile([128, IR], f32)
            t1   = cst.tile([128, IR], f32)
            t2   = cst.tile([128, IR], f32)
            den4 = cst.tile([4, IR], f32)
            rd4  = cst.tile([4, IR], f32)
            rdb  = cst.tile([128, IR], f32)
            outT = cst.tile([128, IR], f32)
            out_sb = cst.tile([128, 8, HF], f32)

            adj_r = adj_d[:].rearrange("(s p) j -> p s j", p=128)

            # ---------------- phase emitters ----------------
            def emit_mask():
                """adj -> binarize -> PE transpose -> mT_all [j, jt, i] fp16."""
                with tc.tile_pool(name="adjp", bufs=2) as adjp, \
                     tc.tile_pool(name="mip", bufs=2) as mip, \
                     tc.tile_pool(name="mtp", bufs=3, space="PSUM") as mtp:
                    for blk in range(8):
                        at = adjp.tile([128, 8, 256], f32, tag="adj")
                        nc.sync.dma_start(at[:], adj_r[:, :, blk * 256:(blk + 1) * 256])
                        mi = mip.tile([128, 8, 256], f16, tag="mi")
                        beng = nc.gpsimd if blk % 2 == 1 else nc.vector
                        beng.tensor_scalar(mi[:], at[:], 0.5, None, op0=Alu.is_gt)
                        for q in range(2):
                            jt = blk * 2 + q
                            for half in range(2):
                                pt = mtp.tile([128, 512], f16, tag="mt")
                                for sub4 in range(4):
                                    sub = half * 4 + sub4
                                    nc.tensor.transpose(
                                        pt[:, sub4 * 128:(sub4 + 1) * 128],
                                        mi[:, sub, q * 128:(q + 1) * 128], idc16[:])
                                if half == 0:
                                    nc.scalar.copy(
                                        mT_all[:, jt, 0:512], pt[:])
                                else:
                                    nc.vector.tensor_copy(
                                        mT_all[:, jt, 512:1024], pt[:])

            def emit_main():
                """G phase, mask matmuls, combine, output."""
                with tc.tile_pool(name="gp", bufs=4) as gp, \
                     tc.tile_pool(name="psg", bufs=2, space="PSUM") as psg, \
                     tc.tile_pool(name="psm", bufs=1, space="PSUM") as psm:
                    pM = psm.tile([128, IR], f32, tag="pm")
                    pMd = psm.tile([128, IR], f32, tag="pmd")
                    for h in range(H):
                        pg = psg.tile([128, IR], f32, tag="pg")
                        for jt in range(NJT):
                            gpre = gp.tile([128, IR], f16, tag="gpre")
                            nc.vector.tensor_scalar(
                                gpre[:], sb16[:, h, :], negt[:, jt, h:h + 1], None,
                                op0=Alu.is_ge)
                            g = gp.tile([128, IR], f16, tag="g")
                            geng = nc.gpsimd if jt in (7, 15) else nc.vector
                            geng.tensor_tensor(g[:], gpre[:], mT_all[:, jt, :],
                                               op=Alu.mult)
                            for k in range(2):
                                nc.tensor.matmul(
                                    pg[:, k * 512:(k + 1) * 512],
                                    Gpack[:, jt, h, :],
                                    g[:, k * 512:(k + 1) * 512],
                                    start=(jt == 0), stop=(jt == NJT - 1))
                            if h == 0:
                                for k in range(2):
                                    nc.tensor.matmul(
                                        pM[:, k * 512:(k + 1) * 512],
                                        Mpack[:, jt, :],
                                        mT_all[:, jt, k * 512:(k + 1) * 512],
                                        start=(jt == 0), stop=(jt == NJT - 1))
                                    nc.tensor.matmul(
                                        pMd[:, k * 512:(k + 1) * 512],
                                        MdenP[:, jt, :],
                                        mT_all[:, jt, k * 512:(k + 1) * 512],
                                        start=(jt == 0), stop=(jt == NJT - 1))
                        # drain this head's psum to SBUF (32-aligned blocks)
                        hs = slice(h * 32, (h + 1) * 32)
                        nc.scalar.copy(As_all[hs, :], pg[0:32, :])
                        nc.scalar.copy(DenA[hs, :], pg[32:64, :])
                        nc.scalar.copy(Cs_all[hs, :], pg[64:96, :])
                        nc.scalar.copy(DenC[hs, :], pg[96:128, :])
                    nc.scalar.copy(Ms_sb[:], pM[:])
                    nc.scalar.copy(Mds_sb[:], pMd[:])

                # combine: den chain on GPSIMD (parallel with num chain on DVE)
                dall = Mds_sb  # reuse
                num = Ms_sb    # reuse
                nc.gpsimd.tensor_tensor(t2[:], Mds_sb[:], DenC[:], op=Alu.add)
                nc.gpsimd.tensor_tensor(t2[:], t2[:], u02b[:], op=Alu.mult)
                nc.gpsimd.tensor_tensor(DenA[:], DenA[:], u1b[:], op=Alu.mult)
                nc.gpsimd.tensor_tensor(dall[:], t2[:], DenA[:], op=Alu.add)
                nc.vector.tensor_tensor(t1[:], Ms_sb[:], Cs_all[:], op=Alu.add)
                nc.vector.tensor_tensor(t1[:], t1[:], u02b[:], op=Alu.mult)
                nc.vector.tensor_tensor(As_all[:], As_all[:], u1b[:], op=Alu.mult)
                nc.vector.tensor_tensor(num[:], t1[:], As_all[:], op=Alu.add)

                with tc.tile_pool(name="fps", bufs=1, space="PSUM") as fps, \
                     tc.tile_pool(name="fpt", bufs=2, space="PSUM") as fpt:
                    pd = fps.tile([4, IR], f32, tag="pd")
                    for k in range(2):
                        nc.tensor.matmul(pd[:, k * 512:(k + 1) * 512], s4c[:],
                                         dall[:, k * 512:(k + 1) * 512],
                                         start=True, stop=True)
                    nc.scalar.copy(den4[:], pd[:])
                    nc.vector.reciprocal_approx_accurate(rd4[:], den4[:], t1[0:4, :])
                    prb = fps.tile([128, IR], f32, tag="prb")
                    for k in range(2):
                        nc.tensor.matmul(prb[:, k * 512:(k + 1) * 512], b4c[:],
                                         rd4[:, k * 512:(k + 1) * 512],
                                         start=True, stop=True)
                    nc.scalar.copy(rdb[:], prb[:])

                    nc.vector.tensor_tensor(outT[:], num[:], rdb[:], op=Alu.mult)
                    nc.vector.tensor_scalar(outT[:], outT[:], biasC[:, 0:1], None,
                                            op0=Alu.add)

                    for sub in range(8):
                        pt = fpt.tile([128, 128], f32, tag="pt")
                        nc.tensor.transpose(pt[:], outT[:, sub * 128:(sub + 1) * 128],
                                            idc[:])
                        nc.scalar.copy(out_sb[:, sub, :], pt[:])
                    nc.sync.dma_start(
                        out_d[:].rearrange("(s p) f -> p s f", p=128), out_sb[:])

            # ---------------- prep ----------------
            with tc.tile_pool(name="pp", bufs=4, space="PSUM") as pp:
                for k in range(4):
                    ps = pp.tile([HF, 512], f32, tag="pp")
                    nc.tensor.matmul(ps[:], Wf[:], xT[:, k * 512:(k + 1) * 512],
                                     start=True, stop=True)
                    nc.scalar.copy(hT[:, k * 512:(k + 1) * 512], ps[:])
                for k in range(2):
                    ps = pp.tile([HF, 512], f32, tag="pp")
                    nc.tensor.matmul(ps[:], Wf[:], xiT[:, k * 512:(k + 1) * 512],
                                     start=True, stop=True)
                    nc.scalar.copy(hiT[:, k * 512:(k + 1) * 512], ps[:])
                for jt in range(NJT):
                    ps = pp.tile([128, HF], f32, tag="pp")
                    nc.tensor.matmul(ps[:], xT[:, jt * 128:(jt + 1) * 128], Wf[:],
                                     start=True, stop=True)
                    nc.scalar.copy(h_sb[:, jt, :], ps[:])
                for k in range(2):
                    ps = pp.tile([4, 512], f32, tag="pp")
                    nc.tensor.matmul(ps[:], aS[:], hiT[:, k * 512:(k + 1) * 512],
                                     start=True, stop=True)
                    nc.scalar.copy(sZ4[:, k * 512:(k + 1) * 512], ps[:])
                for jt in range(NJT):
                    ps = pp.tile([128, H], f32, tag="pp")
                    nc.tensor.matmul(ps[:], hT[:, jt * 128:(jt + 1) * 128], aD[:],
                                     start=True, stop=True)
                    nc.scalar.mul(negt[:, jt, :], ps[:], -1.0)
                    nc.scalar.activation(v1c[:, jt, :], ps[:], Act.Exp)
                    nc.scalar.activation(v02c[:, jt, :], ps[:], Act.Exp, scale=0.2)

                nc.scalar.activation(u1r4[:], sZ4[:], Act.Exp)
                nc.scalar.activation(u02r4[:], sZ4[:], Act.Exp, scale=0.2)
                nc.vector.tensor_copy(s16_4[:], sZ4[:])

                # broadcasts via PE
                for h in range(H):
                    for k in range(2):
                        ps = pp.tile([128, 512], f32, tag="pp")
                        nc.tensor.matmul(ps[:], eyc[:, h * 128:(h + 1) * 128],
                                         s16_4[:, k * 512:(k + 1) * 512],
                                         start=True, stop=True)
                        nc.scalar.copy(sb16[:, h, k * 512:(k + 1) * 512], ps[:])
                for src, dst in ((u1r4, u1b), (u02r4, u02b)):
                    for k in range(2):
                        ps = pp.tile([128, 512], f32, tag="pp")
                        nc.tensor.matmul(ps[:], b4c[:], src[:, k * 512:(k + 1) * 512],
                                         start=True, stop=True)
                        nc.scalar.copy(dst[:, k * 512:(k + 1) * 512], ps[:])

                # mask phase for rep 0 emitted here so binarize/transpose
                # interleave with the prep tail in every engine stream
                emit_mask()

                # ---------------- packs ----------------
                nc.gpsimd.memset(Gpack[:], 0.0)
                nc.gpsimd.memset(MdenP[:], 0.0)
                for h in range(H):
                    hsl = h_sb[:, :, h * 32:(h + 1) * 32]
                    nc.vector.tensor_tensor(
                        Mpack[:, :, h * 32:(h + 1) * 32], hsl,
                        v02c[:, :, h:h + 1].broadcast_to([128, NJT, 32]), op=Alu.mult)
                    nc.vector.tensor_tensor(
                        Gpack[:, :, h, 0:32], hsl,
                        v1c[:, :, h:h + 1].broadcast_to([128, NJT, 32]), op=Alu.mult)
                    nc.vector.tensor_scalar(
                        Gpack[:, :, h, 64:96], Mpack[:, :, h * 32:(h + 1) * 32],
                        -1.0, None, op0=Alu.mult)
                    nc.vector.tensor_copy(Gpack[:, :, h, 32:33], v1c[:, :, h:h + 1])
                    nc.vector.tensor_scalar(
                        Gpack[:, :, h, 96:97], v02c[:, :, h:h + 1], -1.0, None,
                        op0=Alu.mult)
                    nc.vector.tensor_copy(MdenP[:, :, h * 32:h * 32 + 1],
                                          v02c[:, :, h:h + 1])

            prep_ctx.__exit__(None, None, None)

            emit_main()
            for _rep in range(1, reps):
                emit_mask()
                emit_main()
        finally:
            cst_ctx.__exit__(None, None, None)

    nc.compile()
    return nc


def _prepare_in_maps(x, adj, W, a_src, a_dst, bias):
    x = np.ascontiguousarray(np.asarray(x, dtype=np.float32))
    adj = np.asarray(adj, dtype=np.float32)
    W = np.asarray(W, dtype=np.float32)
    a_src = np.asarray(a_src, dtype=np.float32)
    a_dst = np.asarray(a_dst, dtype=np.float32)
    bias = np.asarray(bias, dtype=np.float32)

    Wf = np.ascontiguousarray(W.reshape(D, HF))
    aS = np.zeros((HF, H), np.float32)
    aD = np.zeros((HF, H), np.float32)
    for h in range(H):
        aS[h * F:(h + 1) * F, h] = a_src[h]
        aD[h * F:(h + 1) * F, h] = a_dst[h]
    biasC = np.ascontiguousarray(bias.reshape(HF, 1))

    in_maps = []
    for c in range(NCORES):
        b, cc = c // 2, c % 2
        i0 = cc * IR
        in_maps.append({
            "xT": np.ascontiguousarray(x[b].T),
            "xiT": np.ascontiguousarray(x[b, i0:i0 + IR].T),
            "adjS": np.ascontiguousarray(adj[b, i0:i0 + IR, :]),
            "Wf": Wf,
            "aS": aS,
            "aD": aD,
            "biasC": biasC,
        })
    return in_maps


def run(inputs, trace=False, trace_cores=None):
    from concourse.bass_utils import run_bass_kernel_spmd
    if "nc" not in _CACHE:
        _CACHE["nc"] = build_nc()
    nc = _CACHE["nc"]
    in_maps = _prepare_in_maps(**inputs)
    kw = {}
    if trace:
        kw = dict(trace=True, trace_cores=trace_cores or [0])
    res = run_bass_kernel_spmd(nc, in_maps, list(range(NCORES)), **kw)
    out = np.zeros((B, N, HF), np.float32)
    for c in range(NCORES):
        b, cc = c // 2, c % 2
        out[b, cc * IR:(cc + 1) * IR, :] = res.results[c]["out"]
    return out, res


def kernel(**inputs):
    out, _ = run(inputs, trace=False)
    return out
